# revision 21
# baseline (speedup 1.0000x reference)
"""AlphaRotatedGIoULoss on 8 TRN2 NeuronCores.

Data-parallel: 500000 box pairs sharded 62500/core, laid out as
(125 partitions x 500 boxes). Per-box rotated-GIoU via a branchless
line-integral intersection (slab clipping in each box's axis-aligned
frame + a frame-change correction term), so no sorting/gather is needed.

v4: heavy elementwise chain in fp16 (DVE 2x_1p mode = 2x throughput),
geometry pre-scaled by 1/16 (folded into existing scale factors) so all
products stay in fp16 range; reciprocal slab planes clamped to +-3e4 so
0*inf NaNs cannot occur. Host repack: angle and w/h columns are cast to
fp16 (error ~0.06px, far under tolerance) and shipped as planar rows so
every SBUF slice is packed; xy stays fp32 for exact center differences.
DMA order ang -> wh -> xy unblocks the Sin chain ~2us in. Point-symmetry
(corner e2,e3 = 2*dX - e0,e1) is exploited with stride-0 broadcast APs
to merge op pairs into single wide DVE passes. Enclosing-box extents
use half_extent_x = |wc|+|hs|. Output: per-core sum(giou); host 1-s/N.
"""
import sys
import numpy as np

for _p in ("/opt/trn_rl_repo", "/root/.axon_site/_ro/trn_rl_repo"):
    if _p not in sys.path:
        sys.path.insert(0, _p)

N_CORES = 8
N_TOTAL = 500000
N_CORE = N_TOTAL // N_CORES   # 62500
P = 128                       # all partitions
FB = 489                      # boxes per partition row (128*489 = 62592)
NPAD = P * FB                 # per-core padded count (92 identity pad boxes)
SW = 2 * FB                   # stacked width (both halves)
PI_2 = 1.5707963267948966
SC = 1.0 / 16.0               # global geometry scale (power of 2, exact)
XQ = 32.0                     # xy fixed-point scale (int16 units = px/32)
XSC = SC / XQ                 # folds the xy dequant into the trig scale
CL = 30000.0                  # fp16-safe clamp for reciprocal planes

_CACHE = {}


def _build():
    import concourse.bass as bass
    import concourse.bacc as bacc
    import concourse.tile as tile
    from concourse import mybir

    f32 = mybir.dt.float32
    f16 = mybir.dt.float16
    i16 = mybir.dt.int16
    AF = mybir.ActivationFunctionType
    OP = mybir.AluOpType
    AXL = mybir.AxisListType
    import os
    debug = bool(os.environ.get("K_DEBUG"))
    nc = bacc.Bacc(None, target_bir_lowering=False)
    ang_d = nc.declare_dram_parameter("ang", [P, 2 * FB], f16, isOutput=False)
    wh_d = nc.declare_dram_parameter("wh", [P, 4 * FB], f16, isOutput=False)
    xy_d = nc.declare_dram_parameter("xy", [P, 4 * FB], i16, isOutput=False)
    out_d = nc.declare_dram_parameter("out", [P, 1], f32, isOutput=True)
    dbg_d = None
    if debug:
        dbg_d = nc.declare_dram_parameter("dbg", [4, P, FB], f32, isOutput=True)

    V = nc.vector
    S = nc.scalar

    def vtt(out, a, b, op):
        V.tensor_tensor(out, a, b, op)

    def vts(out, in_, s1, s2, op0, op1=None):
        if op1 is None:
            V.tensor_scalar(out, in_, s1, None, op0)
        else:
            V.tensor_scalar(out, in_, s1, s2, op0, op1)

    def bce(apv, n=2, axis=1):
        # stride-0 broadcast: insert a [0, n] dim at `axis` (after partition)
        ap_l = [list(d) for d in apv.ap]
        ap_l.insert(axis, [0, n])
        return bass.AP(apv.tensor, apv.offset, ap_l)

    from contextlib import ExitStack

    with tile.TileContext(nc) as tc:
        with (
            tc.tile_pool(name="pre", bufs=1) as pre,
            tc.tile_pool(name="small", bufs=1) as sm,
            ExitStack() as stack,
        ):
            io = stack.enter_context(tc.tile_pool(name="io", bufs=1))
            angT = io.tile([P, 2 * FB], f16, tag="angT")
            whT = io.tile([P, 4 * FB], f16, tag="whT")
            xyT = io.tile([P, 4 * FB], i16, tag="xyT")
            pio2 = sm.tile([P, 1], f32, tag="pio2")
            V.memset(pio2[:], PI_2)
            # 1-elem warm-up: loads the Sin ACT table while the DMA runs
            warm = sm.tile([P, 1], f32, tag="warm")
            S.activation(warm[:], pio2[:], AF.Sin)
            angV = angT[:].rearrange("p (h f) -> p h f", h=2)
            whV = whT[:].rearrange("p (c f) -> p c f", c=4)   # w1,w2,h1,h2
            xyV = xyT[:].rearrange("p (c f) -> p c f", c=4)   # x1,x2,y1,y2
            # host pre-shuffles inputs into these exact SBUF layouts, so each
            # partition line is one fully-contiguous DMA descriptor.
            # angles first (small, unblocks the Sin chain), then wh, then xy
            nc.sync.dma_start(out=angT[:], in_=ang_d[:])
            nc.sync.dma_start(out=whT[:], in_=wh_d[:])
            nc.sync.dma_start(out=xyT[:], in_=xy_d[:])

            class SP:
                def __init__(self, name, dt=f16, w=FB, k=2):
                    self.w = w
                    self.t = pre.tile([P, k * w], dt, tag=name)

                def full(self):
                    return self.t[:]

                def h(self, i):
                    return self.t[:, i * self.w:(i + 1) * self.w]

                def v3(self):     # (P, 2, w) stacked view
                    return self.t[:].rearrange("p (h f) -> p h f", h=2)

            # paired tiles (P, 2, SW): two SW-wide planes side by side
            ddS = SP("ddS", w=SW)       # [ddx | ddy]
            cdsd = SP("cdsd", w=SW)     # [cd | sd]
            wcws = SP("wcws", w=SW)     # [wc | ws]
            hchs = SP("hchs", w=SW)     # [hc | hs]
            aP1, aP2 = SP("aP1", w=SW), SP("aP2", w=SW)
            dx16, dy16 = SP("dx16"), SP("dy16")
            dlt, dltw = SP("dlt", f32), SP("dltw", f32)
            cS, sS = SP("cS"), SP("sS")
            csS, ssS = SP("csS"), SP("ssS")
            dX, dY = SP("dX"), SP("dY")
            dXm, dYm = SP("dXm"), SP("dYm")
            whS, hhS = SP("whS"), SP("hhS")
            g0x, g0y, n1, n2 = SP("g0x"), SP("g0y"), SP("n1"), SP("n2")
            Wc, Hc, nWc, nHc = SP("Wc"), SP("Hc"), SP("nWc"), SP("nHc")
            exP, eyP = SP("exP"), SP("eyP")
            rp32a, rp32b = SP("rp32a", f32), SP("rp32b", f32)
            ddxS, ddyS = ddS.v3()[:, 0], ddS.v3()[:, 1]     # (P, SW) each
            cdS, sdS = cdsd.v3()[:, 0], cdsd.v3()[:, 1]
            wcF, wsF = wcws.v3()[:, 0], wcws.v3()[:, 1]
            hcF, hsF = hchs.v3()[:, 0], hchs.v3()[:, 1]

            def hviews(flat):     # (P, 2, FB) of an (P, SW) flat view
                return flat.rearrange("p (h f) -> p h f", h=2)

            # persistent pre-signed clamped reciprocal planes, (P, 2e, 2h, FB)
            rIX = pre.tile([P, 2 * SW], f16, tag="rIX")
            rIY = pre.tile([P, 2 * SW], f16, tag="rIY")
            rIXe = rIX[:].rearrange("p (e h f) -> p e h f", e=2, h=2)
            rIYe = rIY[:].rearrange("p (e h f) -> p e h f", e=2, h=2)

            # ---- pre-pass, angle part (only needs angT) ----
            vtt(dlt.h(0), angV[:, 0], angV[:, 1], OP.subtract)    # a1-a2 (f32)
            vts(dlt.h(1), dlt.h(0), -1.0, None, OP.mult)
            S.activation(cS.h(0), angV[:, 1], AF.Sin, bias=pio2[:])  # c2
            S.activation(cS.h(1), angV[:, 0], AF.Sin, bias=pio2[:])  # c1
            S.activation(sS.h(0), angV[:, 1], AF.Sin)                # s2
            S.activation(sS.h(1), angV[:, 0], AF.Sin)                # s1
            S.activation(sdS, dlt.full(), AF.Sin)                    # [sd|-sd]
            # cos(dlt) = sin(dlt + pi/2); wrap into [-pi, pi] first
            V.add_range_wrap(dltw.full(), dlt.full(), PI_2, 3.141592653589793,
                             6.283185307179586)
            S.activation(cdS, dltw.full(), AF.Sin)                   # [cd|cd]
            # scaled trig copies carry geometry scale + xy dequant into dX/dY
            vts(csS.full(), cS.full(), XSC, None, OP.mult)
            vts(ssS.full(), sS.full(), XSC, None, OP.mult)

            # ---- pre-pass, wh part ----
            vts(whS.full(), whV[:, 0:2], 0.5 * SC, None, OP.mult)  # [w1|w2]/32
            vts(hhS.full(), whV[:, 2:4], 0.5 * SC, None, OP.mult)
            # [wc|ws] = whS * [cd|sd];  [hc|hs] = hhS * [cd|sd]
            cdsd4 = cdsd.t[:].rearrange("p (c h f) -> p c h f", c=2, h=2)
            vtt(wcws.t[:].rearrange("p (c h f) -> p c h f", c=2, h=2),
                bce(whS.v3()), cdsd4, OP.mult)
            vtt(hchs.t[:].rearrange("p (c h f) -> p c h f", c=2, h=2),
                bce(hhS.v3()), cdsd4, OP.mult)
            vtt(g0x.full(), wcF, hsF, OP.subtract)
            vtt(g0y.full(), wsF, hcF, OP.add)
            vtt(n1.full(), wcF, hsF, OP.add)              # -g1x
            vtt(n2.full(), hcF, wsF, OP.subtract)         # g1y
            # clip half-extents of the fixed box, /16 (+neg)
            vts(Wc.h(0), whV[:, 1], 0.5 * SC, None, OP.mult)
            vts(Wc.h(1), whV[:, 0], 0.5 * SC, None, OP.mult)
            vts(Hc.h(0), whV[:, 3], 0.5 * SC, None, OP.mult)
            vts(Hc.h(1), whV[:, 2], 0.5 * SC, None, OP.mult)
            vts(nWc.full(), Wc.full(), -1.0, None, OP.mult)
            vts(nHc.full(), Hc.full(), -1.0, None, OP.mult)
            # moving-box bbox half-extents: ex = |wc|+|hs|, ey = |ws|+|hc|
            S.activation(aP1.full(), wcws.full(), AF.Abs)   # [|wc| | |ws|]
            S.activation(aP2.full(), hchs.full(), AF.Abs)   # [|hc| | |hs|]
            vtt(exP.full(), aP1.v3()[:, 0], aP2.v3()[:, 1], OP.add)
            vtt(eyP.full(), aP1.v3()[:, 1], aP2.v3()[:, 0], OP.add)
            # pre-signed reciprocal slab planes: rIX e0 = -1/(2wc),
            # e1 = +1/(2hs); rIY e0 = -1/(2ws), e1 = -1/(2hc).
            # clamp to +-CL then fp16 so 0*inf NaNs cannot occur.
            for (dst, src, sgn, rp) in (
                (rIXe[:, 0], wcF, -1.0, rp32a),
                (rIXe[:, 1], hsF, 1.0, rp32b),
                (rIYe[:, 0], wsF, -1.0, rp32a),
                (rIYe[:, 1], hcF, -1.0, rp32b),
            ):
                vts(rp.full(), src, 2.0 * sgn, 1e-20 * sgn, OP.mult, OP.add)
                V.reciprocal_approx_fast(out=rp.full(), in_=rp.full())
                vts(dst, rp.v3(), CL, -CL, OP.min, OP.max)
            # union0 = (w1h1 + w2h2)/1024; the *4 to reach the /256 scale of
            # inter is folded into the final union STT
            u01 = sm.tile([P, SW], f16, tag="u01")
            union0 = sm.tile([P, FB], f32, tag="union0")
            vtt(u01[:], whS.full(), hhS.full(), OP.mult)
            u013 = u01[:].rearrange("p (h f) -> p h f", h=2)
            vtt(union0[:], u013[:, 0], u013[:, 1], OP.add)

            # ---- pre-pass, xy part (lands last) ----
            vtt(hviews(ddxS)[:, 0], xyV[:, 0], xyV[:, 1], OP.subtract)  # x1-x2
            vts(hviews(ddxS)[:, 1], hviews(ddxS)[:, 0], -1.0, None, OP.mult)
            vtt(hviews(ddyS)[:, 0], xyV[:, 2], xyV[:, 3], OP.subtract)
            vts(hviews(ddyS)[:, 1], hviews(ddyS)[:, 0], -1.0, None, OP.mult)
            vts(dx16.full(), ddxS, XSC, None, OP.mult)
            vts(dy16.full(), ddyS, XSC, None, OP.mult)
            # delta = R^T * (center difference)/16, stacked:
            # P1 = [csS*ddx | csS*ddy], P2 = [ssS*ddx | ssS*ddy]
            ddc = ddS.t[:].rearrange("p (c h f) -> p c h f", c=2, h=2)
            vtt(aP1.t[:].rearrange("p (c h f) -> p c h f", c=2, h=2),
                bce(csS.v3()), ddc, OP.mult)
            vtt(aP2.t[:].rearrange("p (c h f) -> p c h f", c=2, h=2),
                bce(ssS.v3()), ddc, OP.mult)
            vtt(dX.full(), aP1.v3()[:, 0], aP2.v3()[:, 1], OP.add)
            vtt(dY.full(), aP1.v3()[:, 1], aP2.v3()[:, 0], OP.subtract)
            vts(dXm.full(), dX.full(), 2.0, None, OP.mult)        # 2*dx
            vts(dYm.full(), dY.full(), 2.0, None, OP.mult)

            # input tiles no longer needed: free the io pool
            stack.close()
            hv = stack.enter_context(tc.tile_pool(name="heavy", bufs=1))

            def E(tile4):     # (P, 4, 2, FB) edge/half view of 4*SW tile
                return tile4[:].rearrange("p (e h f) -> p e h f", e=4, h=2)

            AXt = hv.tile([P, 4 * SW], f16, tag="AXt")
            AYt = hv.tile([P, 4 * SW], f16, tag="AYt")
            DRX = hv.tile([P, 4 * SW], f16, tag="DRX")
            DRY = hv.tile([P, 4 * SW], f16, tag="DRY")
            Ut = hv.tile([P, 4 * SW], f16, tag="Ut")
            Vt = hv.tile([P, 4 * SW], f16, tag="Vt")
            NPt = hv.tile([P, 4 * SW], f16, tag="NPt")
            TLX = hv.tile([P, 4 * SW], f16, tag="TLX")

            # corners: e0,e1 explicit; e2,e3 = 2*dX - e0,e1 (point symmetry)
            vtt(E(AXt)[:, 0], dX.v3(), g0x.v3(), OP.add)
            vtt(E(AXt)[:, 1], dX.v3(), n1.v3(), OP.subtract)
            vtt(E(AXt)[:, 2:4], bce(dXm.v3()), E(AXt)[:, 0:2], OP.subtract)
            vtt(E(AYt)[:, 0], dY.v3(), g0y.v3(), OP.add)
            vtt(E(AYt)[:, 1], dY.v3(), n2.v3(), OP.add)
            vtt(E(AYt)[:, 2:4], bce(dYm.v3()), E(AYt)[:, 0:2], OP.subtract)

            # ---- enclosing rect (bbox in each frame, min of the two) ----
            exm = sm.tile([P, SW], f16, tag="exm")
            exn = sm.tile([P, SW], f16, tag="exn")
            exs = sm.tile([P, SW], f16, tag="exs")
            eys = sm.tile([P, SW], f16, tag="eys")
            ex3 = exm[:].rearrange("p (h f) -> p h f", h=2)
            en3 = exn[:].rearrange("p (h f) -> p h f", h=2)
            es3 = exs[:].rearrange("p (h f) -> p h f", h=2)
            ey3 = eys[:].rearrange("p (h f) -> p h f", h=2)
            for ext, d3, clamp, dst3 in ((exP, dX, Wc, es3), (eyP, dY, Hc, ey3)):
                vtt(ex3, d3.v3(), ext.v3(), OP.add)               # dX + ex
                vtt(en3, ext.v3(), d3.v3(), OP.subtract)          # ex - dX
                vtt(ex3, ex3, clamp.v3(), OP.max)
                vtt(en3, en3, clamp.v3(), OP.max)
                vtt(dst3, ex3, en3, OP.add)                       # extent
            vtt(exs[:], exs[:], eys[:], OP.mult)                  # areaC stacked
            area_c = sm.tile([P, FB], f32, tag="area_c")
            vtt(area_c[:], es3[:, 0], es3[:, 1], OP.min)

            HW2 = 2 * SW

            def H01(t4):
                return t4[:, 0:HW2]

            def H23(t4):
                return t4[:, HW2:2 * HW2]

            # ---- slab clip, x axis, edges 0,1 (2,3 via point symmetry:
            # roots(edge2) = m + roots(edge0), m = dXm*rIX) ----
            vtt(E(Ut)[:, 0:2], bce(nWc.v3()), E(AXt)[:, 0:2], OP.subtract)
            vtt(E(Vt)[:, 0:2], bce(Wc.v3()), E(AXt)[:, 0:2], OP.subtract)
            vtt(H01(Ut), H01(Ut), rIX[:], OP.mult)                 # ta01
            vtt(H01(Vt), H01(Vt), rIX[:], OP.mult)                 # tb01
            vtt(H01(TLX), H01(Ut), H01(Vt), OP.min)                # tlo01
            vtt(H01(Ut), H01(Ut), H01(Vt), OP.max)                 # thi01
            vtt(E(Vt)[:, 0:2], bce(dXm.v3()), rIXe, OP.mult)       # m01
            vtt(H23(TLX), H01(Vt), H01(TLX), OP.add)               # tlo23
            vtt(H23(Ut), H01(Vt), H01(Ut), OP.add)                 # thi23
            # ---- slab clip, y axis, edges 0,1 ----
            vtt(E(Vt)[:, 0:2], bce(nHc.v3()), E(AYt)[:, 0:2], OP.subtract)
            vtt(E(NPt)[:, 0:2], bce(Hc.v3()), E(AYt)[:, 0:2], OP.subtract)
            vtt(H01(Vt), H01(Vt), rIY[:], OP.mult)                 # ta01_y
            vtt(H01(NPt), H01(NPt), rIY[:], OP.mult)               # tb01_y
            vtt(H01(DRX), H01(Vt), H01(NPt), OP.min)               # tlo01_y
            vtt(H01(Vt), H01(Vt), H01(NPt), OP.max)                # thi01_y
            vtt(E(NPt)[:, 0:2], bce(dYm.v3()), rIYe, OP.mult)      # m01_y
            vtt(H23(DRX), H01(NPt), H01(DRX), OP.add)              # tlo23_y
            vtt(H23(Vt), H01(NPt), H01(Vt), OP.add)                # thi23_y
            # ---- interval intersect, dt ----
            # t0 = max(tlo_x, tlo_y, 0); t1 = min(thi_x, thi_y, 1)
            vtt(TLX[:], TLX[:], DRX[:], OP.max)
            vts(TLX[:], TLX[:], 0.0, None, OP.max)
            vtt(Ut[:], Ut[:], Vt[:], OP.min)
            vts(Ut[:], Ut[:], 1.0, None, OP.min)
            vtt(TLX[:], Ut[:], TLX[:], OP.subtract)                # t1-t0
            S.activation(TLX[:], TLX[:], AF.Relu)                  # dt
            # ---- direction planes (on Scalar), cross(a,d), pieces ----
            for dst, srcs in (
                (DRX, ((wcF, -2.0), (hsF, 2.0), (wcF, 2.0), (hsF, -2.0))),
                (DRY, ((wsF, -2.0), (hcF, -2.0), (wsF, 2.0), (hcF, 2.0))),
            ):
                d4 = E(dst)
                for e, (src, sc) in enumerate(srcs):
                    S.activation(d4[:, e], hviews(src), AF.Copy, scale=sc)
            vtt(Vt[:], AXt[:], DRY[:], OP.mult)                    # ax*dy
            vtt(NPt[:], AYt[:], DRX[:], OP.mult)                   # ay*dx
            vtt(Vt[:], Vt[:], NPt[:], OP.subtract)                 # cad
            vtt(Ut[:], TLX[:], Vt[:], OP.mult)                     # pieces

            # ---- piece sum (stacked), SA correction (frame-B half) ----
            psS = sm.tile([P, SW], f16, tag="psS")
            ps3 = psS[:].rearrange("p (h f) -> p h f", h=2)
            u4 = E(Ut)
            vtt(ps3, u4[:, 0], u4[:, 1], OP.add)
            vtt(es3, u4[:, 2], u4[:, 3], OP.add)                   # reuse exs
            vtt(ps3, ps3, es3, OP.add)
            dt4 = E(TLX)
            sax = sm.tile([P, FB], f16, tag="sax")
            say = sm.tile([P, FB], f16, tag="say")
            sau = sm.tile([P, FB], f16, tag="sau")
            sav = sm.tile([P, FB], f16, tag="sav")
            st1 = sm.tile([P, FB], f16, tag="st1")
            vtt(sau[:], dt4[:, 2, 0], dt4[:, 0, 0], OP.subtract)
            vtt(sav[:], dt4[:, 3, 0], dt4[:, 1, 0], OP.subtract)
            dx4, dy4 = E(DRX), E(DRY)
            vtt(sax[:], dx4[:, 2, 0], sau[:], OP.mult)
            vtt(st1[:], dx4[:, 3, 0], sav[:], OP.mult)
            vtt(sax[:], sax[:], st1[:], OP.add)
            vtt(say[:], dy4[:, 2, 0], sau[:], OP.mult)
            vtt(st1[:], dy4[:, 3, 0], sav[:], OP.mult)
            vtt(say[:], say[:], st1[:], OP.add)
            # corr = ddy/16*(c2*sax - s2*say) - ddx/16*(s2*sax + c2*say)
            c2v = cS.h(0)
            s2v = sS.h(0)
            rsx = sm.tile([P, FB], f16, tag="rsx")
            rsy = sm.tile([P, FB], f16, tag="rsy")
            vtt(rsx[:], c2v, sax[:], OP.mult)
            vtt(st1[:], s2v, say[:], OP.mult)
            vtt(rsx[:], rsx[:], st1[:], OP.subtract)
            vtt(rsy[:], s2v, sax[:], OP.mult)
            vtt(st1[:], c2v, say[:], OP.mult)
            vtt(rsy[:], rsy[:], st1[:], OP.add)
            inter16 = sm.tile([P, FB], f16, tag="inter16")
            vtt(inter16[:], dy16.h(0), rsx[:], OP.mult)
            vtt(st1[:], dx16.h(0), rsy[:], OP.mult)
            vtt(inter16[:], inter16[:], st1[:], OP.subtract)       # corr
            vtt(inter16[:], inter16[:], ps3[:, 0], OP.add)
            vtt(inter16[:], inter16[:], ps3[:, 1], OP.add)
            inter = sm.tile([P, FB], f32, tag="inter")
            S.activation(inter[:], inter16[:], AF.Relu, scale=0.5)  # inter area

            # ---- final loss (fp32) ----
            union = sm.tile([P, FB], f32, tag="union")
            fr1 = sm.tile([P, FB], f32, tag="fr1")
            iou = sm.tile([P, FB], f32, tag="iou")
            rr = sm.tile([P, FB], f32, tag="rr")
            lsa = sm.tile([P, 1], f32, tag="lsa")
            # union = 4*union0 - inter  (the *4 restores the /256 scale)
            V.scalar_tensor_tensor(union[:], union0[:], 4.0, inter[:],
                                   OP.mult, OP.subtract)
            V.reciprocal_approx_fast(out=fr1[:], in_=union[:])
            vtt(iou[:], inter[:], fr1[:], OP.mult)
            vts(iou[:], iou[:], 1e-6, None, OP.max)
            V.reciprocal_approx_fast(out=fr1[:], in_=area_c[:])
            vtt(fr1[:], union[:], fr1[:], OP.mult)
            vts(rr[:], fr1[:], -1.0, 1.0, OP.mult, OP.add)         # 1 - u/ac
            vtt(fr1[:], iou[:], iou[:], OP.mult)                   # iou^2
            vtt(fr1[:], fr1[:], iou[:], OP.mult)                   # iou^3
            vtt(iou[:], rr[:], rr[:], OP.mult)                     # rr^2
            vtt(iou[:], iou[:], rr[:], OP.mult)                    # rr^3
            vtt(fr1[:], fr1[:], iou[:], OP.subtract)               # giou
            V.tensor_reduce(lsa[:], fr1[:], AXL.X, OP.add)         # sum giou
            if debug:
                nc.sync.dma_start(out=dbg_d[0], in_=fr1[:])
                nc.sync.dma_start(out=dbg_d[1], in_=inter[:])
                nc.sync.dma_start(out=dbg_d[2], in_=union[:])
                nc.sync.dma_start(out=dbg_d[3], in_=area_c[:])
            nc.sync.dma_start(out=out_d[:], in_=lsa[:])

    nc.finalize()
    return nc


def _get_nc():
    if "nc" not in _CACHE:
        _CACHE["nc"] = _build()
    return _CACHE["nc"]


def _repack(pred, target):
    """Per-core input repack: planar rows so every SBUF slice is packed.
    ang/wh in fp16; xy quantized to int16 units of 1/32 px (diffs <= ~1500
    units stay exact in fp16). Rows beyond N_CORE are padded with identity
    boxes (w=h=16, a=0, same centers) whose giou is exactly 1."""
    in_maps = []
    for i in range(N_CORES):
        sl = slice(i * N_CORE, (i + 1) * N_CORE)
        p, t = pred[sl], target[sl]
        ang = np.zeros((2, NPAD), np.float16)
        ang[0, :N_CORE] = p[:, 4]
        ang[1, :N_CORE] = t[:, 4]
        # pads: concentric axis-aligned 16-box (pred) vs 8-box (target):
        # iou = 1/4, rr = 0 -> giou = 1/64 exactly (all fp16-exact values;
        # identical boxes would hit the coincident-boundary degeneracy)
        wh = np.empty((4, NPAD), np.float16)
        wh[0, N_CORE:] = 16.0
        wh[1, N_CORE:] = 8.0
        wh[2, N_CORE:] = 16.0
        wh[3, N_CORE:] = 8.0
        wh[0, :N_CORE] = p[:, 2]
        wh[1, :N_CORE] = t[:, 2]
        wh[2, :N_CORE] = p[:, 3]
        wh[3, :N_CORE] = t[:, 3]
        xy = np.full((4, NPAD), 16384, np.int16)
        for r, col in enumerate((p[:, 0], t[:, 0], p[:, 1], t[:, 1])):
            xy[r, :N_CORE] = np.clip(np.rint(col * XQ), 0, 32767).astype(np.int16)
        # shuffle each (k, NPAD) row-plane set into the SBUF tile layout
        # (P, k*FB): partition-contiguous single-descriptor DMA lines
        def lay(a):
            k = a.shape[0]
            return np.ascontiguousarray(
                a.reshape(k, P, FB).transpose(1, 0, 2).reshape(P, k * FB))
        in_maps.append({"ang": lay(ang), "wh": lay(wh), "xy": lay(xy)})
    return in_maps


def kernel(pred, target):
    from concourse.bass_utils import run_bass_kernel_spmd

    pred = np.ascontiguousarray(np.asarray(pred, dtype=np.float32))
    target = np.ascontiguousarray(np.asarray(target, dtype=np.float32))
    nc = _get_nc()
    in_maps = _repack(pred, target)
    res = run_bass_kernel_spmd(nc, in_maps, core_ids=list(range(N_CORES)))
    gsum = np.float64(0.0)
    for i in range(N_CORES):
        gsum += np.asarray(res.results[i]["out"], dtype=np.float64).sum()
    # subtract the exact giou (=1/64) of the concentric pad boxes
    gsum -= float((NPAD - N_CORE) * N_CORES) * 0.015625
    # loss = mean(1 - giou) = 1 - sum(giou)/N
    return np.float32(1.0 - gsum / N_TOTAL)
